# revision 41
# baseline (speedup 1.0000x reference)
"""Trainium2 Bass kernel for nn_CoPredictor (biaffine co-predictor).

Math (per batch b, class k):
    h    = gelu(x_b @ mlp1_w.T + b1)          (512,256)
    t    = gelu(y_b @ mlp2_w.T + b2)          (512,256)
    head = lrelu(x_b @ head_w.T + bh, .01)    (512,256)
    tail = lrelu(x_b @ tail_w.T + bt, .01)    (512,256)
    out[b,k,m,n] = (h A_k t^T)[m,n] + r[k,m] + cv[k,n] + T0[k, clip(n-m+15,0,29)]
  with A_k = biaf_W[k,:256,:256],
       r[k,m]  = h[m].u_k + head[m].Wh_k          (u_k = biaf_W[k,:256,256], Wh_k = W[k,:256])
       cv[k,n] = t[n].v_k + tail[n].Wt_k          (v_k = biaf_W[k,256,:256], Wt_k = W[k,257:513])
       T0[k,d] = size_emb[d].Ws_k + W[k,256] + W[k,513] + biaf_W[k,256,256]
(z is unused by the reference.)

Sharding: 8 cores = batch(2) x m-half(2) x class-half(2x7).  Each core computes
out[b, k0:k0+7, m0:m0+256, :].  All matmuls run in bf16.  The n axis is
processed reversed (n' = 511-n) on device so the Toeplitz T0 term becomes a
function of (p + n'); the host pre-expands the per-class (128,640) windows and
flips n back at the end.  All inputs are host-packed into their exact SBUF
[128, cols] layouts so every DMA is a dense 2D copy (128 contiguous
descriptors).  PSUM evacuation is one DVE tensor_tensor per class:
out = psum + toeplitz_window; the r row and cv column terms ride a K=2
rank-2 matmul (r x ones + ones x cv) accumulated into the same PSUM group.

Steady-state pipelining (the reps>0 measurement path):
 - For_i has an ALL-ENGINE BARRIER per iteration (~11us of pipeline
   drain+refill), so UNROLL=8 bodies are emitted per iteration.  U=16
   crashes the exec unit (descriptor-ring overflow) - do not raise it.
 - Input tiles are triple-buffered, computed tiles double-buffered, so
   body k+1's input DMAs overlap body k's compute.
 - Queue discipline: SP carries ONLY input loads (first-consumption
   order), outputs ride the Pool/SWDGE queue behind the mid-body rr/cvo
   transfers, Act carries the tiny early loads + activations/copies.
   This keeps body k+1's input descriptor-gen from queueing behind body
   k's late output DMAs.
Measured on trn2 (8 cores, differential-reps): ~32.0us/iter, vs a
22.1us compute-only floor, 20.1us DMA-only floor (6.9MB/body at
~345GB/s), and ~29.8us for independent compute+DMA streams - i.e. the
remaining gap over the floors is HW contention between concurrent DMA
traffic and the compute engines, not dependency stalls.
"""
import sys

sys.path.insert(0, "/opt/trn_rl_repo")

import numpy as np

B, N, HID = 2, 512, 768
BIAF, CLS = 256, 14
KH = 7          # classes per core
P = 128
UNROLL = 8      # bodies per For_i iteration (the loop has an all-engine barrier)

MLP_DT = "bf16"    # dtype of the MLP-stage matmuls (weights + x/y activations)
MAIN_DT = "bf16"   # dtype of the biaffine-stage matmuls
OUT_DT = "bf16"    # dtype of the output tiles / DMA

_nc = {}


def _build_program(act_mode="hw", reps=0, mlp_dt=None, main_dt=None, mode="full",
                   unroll=None, split_in=False, qplan="pool_out", ka=False,
                   mtsplit=False, fewin=True):
    unroll = unroll or UNROLL
    import concourse.bass as bass
    import concourse.bacc as bacc
    import concourse.mybir as mybir
    import concourse.tile as tile

    mlp_dt = mlp_dt or MLP_DT
    main_dt = main_dt or MAIN_DT
    F32 = mybir.dt.float32
    BF16 = mybir.dt.bfloat16
    DT_MLP = mybir.dt.float32r if mlp_dt == "f32r" else BF16
    DT_MAIN = mybir.dt.float32r if main_dt == "f32r" else BF16
    DT_OUT = F32 if OUT_DT == "f32" else BF16
    ADD = mybir.AluOpType.add
    if act_mode == "hw":
        GELU = mybir.ActivationFunctionType.Gelu
        LRELU = mybir.ActivationFunctionType.Prelu   # parametric_relu, same table set as gelu
    else:  # CoreSim doesn't implement Gelu/Prelu; substitute for structure validation
        GELU = mybir.ActivationFunctionType.Tanh
        LRELU = mybir.ActivationFunctionType.Relu

    nc = bacc.Bacc("TRN2", target_bir_lowering=False, debug=False, num_devices=8)

    # All inputs host-packed to the exact SBUF layout: dense [.., P, cols].
    wm = nc.dram_tensor("wm", [4, P, 6 * 256], DT_MLP, kind="ExternalInput").ap()
    xtc = nc.dram_tensor("xtc", [P, 6 * 256], DT_MLP, kind="ExternalInput").ap()
    ytr = nc.dram_tensor("ytr", [P, 6 * 512], DT_MLP, kind="ExternalInput").ap()
    xtr = nc.dram_tensor("xtr", [P, 6 * 512], DT_MLP, kind="ExternalInput").ap()
    abig = nc.dram_tensor("abig", [P, KH * 2 * 2 * P], DT_MAIN, kind="ExternalInput").ap()
    uvw = nc.dram_tensor("uvw", [P, 4 * 2 * 8], DT_MAIN, kind="ExternalInput").ap()
    biasv = nc.dram_tensor("biasv", [P, 8], F32, kind="ExternalInput").ap()
    F8 = mybir.dt.float8e4
    hh = nc.dram_tensor("hh", [P, KH * 640], F8, kind="ExternalInput").ap()
    onesd = nc.dram_tensor("onesd", [1, KH * 512], DT_MAIN, kind="ExternalInput").ap()
    out_d = nc.dram_tensor("out", [KH, 2, P, N], DT_OUT, kind="ExternalOutput").ap()

    with tile.TileContext(nc) as tc:
        with tc.tile_pool(name="const", bufs=6) as cp, \
             tc.tile_pool(name="work", bufs=8) as wp, \
             tc.tile_pool(name="psa", bufs=2, space="PSUM") as psa, \
             tc.tile_pool(name="psg", bufs=2, space="PSUM") as psg, \
             tc.tile_pool(name="psm", bufs=4 if mtsplit else 2,
                          space="PSUM") as psm:

            ka_ones = cp.tile([1, P], DT_MAIN, tag="ka_ones", bufs=1,
                              name="ka_ones")
            nc.vector.memset(ka_ones[:, :], 1.0)

            def ka_mm(pt):
                # HAM keepalive: a tiny dependency-light matmul into the
                # group's own PSUM tile (overwritten by the real start=True
                # matmul) so the PE never shows a fully-idle activity
                # window while waiting for input DMAs.
                nc.tensor.matmul(pt[:, 0:128], ka_ones[0:1, :],
                                 ka_ones[0:1, 0:1].broadcast_to((1, 128)),
                                 start=True, stop=True, skip_group_check=True)

            def warmup():
                # PE warm-up: dependency-free dummy matmuls keep the PE busy
                # from t~0 so its clock is fully ramped when stage A arrives.
                # Emitted once (outside the reps loop): in steady state the
                # PE never idles long enough for the HAM to re-throttle.
                ones_w = cp.tile([1, P], DT_MAIN, tag="ones_w", bufs=1)
                nc.vector.memset(ones_w[:, :], 1.0)
                warm_ps = psg.tile([P, 512], mybir.dt.float32, tag="g")
                for _ in range(4):
                    nc.tensor.matmul(warm_ps[:, :], ones_w[0:1, :],
                                     ones_w[0:1, 0:1].broadcast_to((1, 512)),
                                     start=True, stop=True)

            def body_dma_few(_iv=None):
                # diagnostic: same input/output bytes, one dma_start per tensor
                xtc_sb = cp.tile([P, 6 * 256], DT_MLP, tag="xtc", bufs=3)
                nc.sync.dma_start(out=xtc_sb[:, :], in_=xtc)
                ytr_sb = cp.tile([P, 6 * 512], DT_MLP, tag="ytr")
                nc.sync.dma_start(out=ytr_sb[:, :], in_=ytr)
                xtr_sb = cp.tile([P, 6 * 512], DT_MLP, tag="xtr")
                nc.sync.dma_start(out=xtr_sb[:, :], in_=xtr)
                wmall_sb = cp.tile([P, 4 * 6 * 256], DT_MLP, tag="wmall")
                nc.sync.dma_start(
                    out=wmall_sb[:, :].rearrange("p (w c) -> p w c", w=4),
                    in_=wm.rearrange("w p c -> p w c"))
                a_sb = cp.tile([P, KH * 2 * 2 * 128], DT_MAIN, tag="a")
                nc.sync.dma_start(out=a_sb[:, :], in_=abig)
                tlb_all = cp.tile([P, KH * 640], F8, tag="tlall")
                nc.sync.dma_start(out=tlb_all[:, :], in_=hh)
                for kk in range(KH):
                    nc.gpsimd.dma_start(
                        out=out_d[kk].rearrange("mt p n -> p mt n"),
                        in_=osb_const[:, :].rearrange("p (mt n) -> p mt n", mt=2))

            def body_dma_only(_iv=None):
                # diagnostic: input + output DMAs only (no compute)
                bias_sb = cp.tile([P, 8], F32, tag="bias")
                nc.scalar.dma_start(out=bias_sb[:, :], in_=biasv)
                uvw_sb = cp.tile([P, 4 * 2 * 8], DT_MAIN, tag="uvw")
                nc.scalar.dma_start(out=uvw_sb[:, :], in_=uvw)
                xtc_sb = cp.tile([P, 6 * 256], DT_MLP, tag="xtc")
                nc.sync.dma_start(out=xtc_sb[:, :256], in_=xtc[:, :256])
                nc.sync.dma_start(out=xtc_sb[:, 256:], in_=xtc[:, 256:])
                ytr_sb = cp.tile([P, 6 * 512], DT_MLP, tag="ytr")
                nc.sync.dma_start(out=ytr_sb[:, :2 * 512], in_=ytr[:, :2 * 512])
                nc.sync.dma_start(out=ytr_sb[:, 2 * 512:], in_=ytr[:, 2 * 512:])
                xtr_sb = cp.tile([P, 6 * 512], DT_MLP, tag="xtr")
                nc.sync.dma_start(out=xtr_sb[:, :], in_=xtr)
                wm_sbs = []
                for wi in range(4):
                    wmt = cp.tile([P, 6 * 256], DT_MLP, tag=f"wm{wi}")
                    wm_sbs.append(wmt)
                nc.gpsimd.dma_start(out=wm_sbs[0][:, :256], in_=wm[0][:, :256])
                nc.gpsimd.dma_start(out=wm_sbs[0][:, 256:512], in_=wm[0][:, 256:512])
                nc.gpsimd.dma_start(out=wm_sbs[0][:, 512:], in_=wm[0][:, 512:])
                nc.gpsimd.dma_start(out=wm_sbs[2][:, :], in_=wm[2])
                nc.gpsimd.dma_start(out=wm_sbs[1][:, :], in_=wm[1])
                nc.gpsimd.dma_start(out=wm_sbs[3][:, :], in_=wm[3])
                a_sb = cp.tile([P, KH * 2 * 2 * 128], DT_MAIN, tag="a")
                nc.gpsimd.dma_start(out=a_sb[:, :], in_=abig)
                tlb_all = cp.tile([P, KH * 640], F8, tag="tlall")
                nc.gpsimd.dma_start(out=tlb_all[:, :], in_=hh)
                for kk in range(KH):
                    oeng = nc.scalar if kk % 2 == 0 else nc.sync
                    oeng.dma_start(
                        out=out_d[kk].rearrange("mt p n -> p mt n"),
                        in_=osb_const[:, :].rearrange("p (mt n) -> p mt n", mt=2))

            shared_in = {}

            def alloc_shared_inputs():
                # compute-only diagnostic: input tiles memset once in the
                # prologue, shared by every body (no input DMA traffic).
                t = shared_in
                t["bias"] = cp.tile([P, 8], F32, tag="bias", bufs=1, name="sbias")
                t["uvw"] = cp.tile([P, 64], DT_MAIN, tag="uvw", bufs=1, name="suvw")
                t["xtc"] = cp.tile([P, 6 * 256], DT_MLP, tag="xtc", bufs=1, name="sxtc")
                t["ytr"] = cp.tile([P, 6 * 512], DT_MLP, tag="ytr", bufs=1, name="sytr")
                t["xtr"] = cp.tile([P, 6 * 512], DT_MLP, tag="xtr", bufs=1, name="sxtr")
                t["wm"] = [cp.tile([P, 6 * 256], DT_MLP, tag=f"wm{wi}", bufs=1,
                                   name=f"swm{wi}") for wi in range(4)]
                t["a"] = cp.tile([P, KH * 2 * 2 * 128], DT_MAIN, tag="a", bufs=1,
                                 name="sa")
                t["tlb"] = cp.tile([P, KH * 640], F8, tag="tlall", bufs=1,
                                   name="stlb")
                engs = [nc.vector, nc.gpsimd]
                i = 0
                for ap in [t["bias"], t["uvw"], t["xtc"], t["ytr"], t["xtr"],
                           t["a"], t["tlb"]] + t["wm"]:
                    engs[i % 2].memset(ap[:, :], 0.25)
                    i += 1

            def body(_iv=None):
                # ---- input DMAs, spread across queues, ordered by first consumer ----
                # Act queue: tiny bias vector + uvw first (first activation
                # needs the bias); the big toeplitz table is issued later,
                # after stage A, so it doesn't delay the wm3/abig loads.
                if mode in ("compute", "both"):
                    bias_sb = shared_in["bias"]
                    uvw_sb = shared_in["uvw"]
                    xtc_sb = shared_in["xtc"]
                    ytr_sb = shared_in["ytr"]
                    xtr_sb = shared_in["xtr"]
                    wm_sbs = shared_in["wm"]
                    a_sb = shared_in["a"]
                    tlb_all = shared_in["tlb"]
                    if mode == "both":
                        # same DMA traffic, but into tiles nothing reads:
                        # separates bandwidth/queue contention from RAW
                        # dependency stalls.
                        dxt = cp.tile([P, 6 * 256], DT_MLP, tag="dxtc", bufs=2,
                                      name="dxt")
                        nc.sync.dma_start(out=dxt[:, :256], in_=xtc[:, :256])
                        nc.sync.dma_start(out=dxt[:, 256:], in_=xtc[:, 256:])
                        dyt = cp.tile([P, 6 * 512], DT_MLP, tag="dytr", bufs=2,
                                      name="dyt")
                        nc.sync.dma_start(out=dyt[:, :2 * 512], in_=ytr[:, :2 * 512])
                        nc.sync.dma_start(out=dyt[:, 2 * 512:], in_=ytr[:, 2 * 512:])
                        dxr = cp.tile([P, 6 * 512], DT_MLP, tag="dxtr", bufs=2,
                                      name="dxr")
                        nc.sync.dma_start(out=dxr[:, :], in_=xtr)
                        for wi in range(4):
                            dwm = cp.tile([P, 6 * 256], DT_MLP, tag=f"dwm{wi}",
                                          bufs=2, name=f"dwm{wi}")
                            nc.sync.dma_start(out=dwm[:, :], in_=wm[wi])
                        da = cp.tile([P, KH * 2 * 2 * 128], DT_MAIN, tag="da",
                                     bufs=3, name="da")
                        nc.sync.dma_start(out=da[:, :], in_=abig)
                        dtl = cp.tile([P, KH * 640], F8, tag="dtlall", bufs=2,
                                      name="dtl")
                        nc.sync.dma_start(out=dtl[:, :], in_=hh)
                else:
                    bias_sb = cp.tile([P, 8], F32, tag="bias", bufs=3)
                    uvw_sb = cp.tile([P, 4 * 2 * 8], DT_MAIN, tag="uvw", bufs=3)
                    xtc_sb = cp.tile([P, 6 * 256], DT_MLP, tag="xtc", bufs=3)
                    ytr_sb = cp.tile([P, 6 * 512], DT_MLP, tag="ytr", bufs=3)
                    xtr_sb = cp.tile([P, 6 * 512], DT_MLP, tag="xtr", bufs=3)
                    a_sb = cp.tile([P, KH * 2 * 2 * 128], DT_MAIN, tag="a", bufs=3)
                    tlb_all = cp.tile([P, KH * 640], F8, tag="tlall", bufs=3)
                    nc.scalar.dma_start(out=bias_sb[:, :], in_=biasv)
                    nc.scalar.dma_start(out=uvw_sb[:, :], in_=uvw)
                    # SP (and with split_in also Act) are PURE-INPUT queues
                    # (outputs live on the Pool queue), so body k+1's input
                    # descriptor-gen is never FIFO-blocked behind body k's
                    # late output DMAs.  First-consumption order per queue.
                    if split_in:
                        inq2 = nc.scalar
                    elif qplan == "split_pool":
                        inq2 = nc.gpsimd
                    else:
                        inq2 = nc.sync
                    if fewin:
                        # fewer, larger input DMAs: less per-dma completion
                        # and semaphore overhead on the shared DMA path.
                        wmall = cp.tile([P, 4 * 6 * 256], DT_MLP, tag="wmall",
                                        bufs=3, name="wmall")
                        wm_sbs = [wmall[:, wi * 1536:(wi + 1) * 1536]
                                  for wi in range(4)]
                        nc.sync.dma_start(out=xtc_sb[:, :], in_=xtc)
                        nc.sync.dma_start(
                            out=wmall[:, :].rearrange("p (w c) -> p w c", w=4),
                            in_=wm.rearrange("w p c -> p w c"))
                        inq2.dma_start(out=ytr_sb[:, :], in_=ytr)
                        nc.sync.dma_start(out=xtr_sb[:, :], in_=xtr)
                        nc.sync.dma_start(out=a_sb[:, :], in_=abig)
                        inq2.dma_start(out=tlb_all[:, :], in_=hh)
                    else:
                        wm_sbs = []
                        for wi in range(4):
                            wmt = cp.tile([P, 6 * 256], DT_MLP, tag=f"wm{wi}",
                                          bufs=3)
                            wm_sbs.append(wmt)
                        nc.sync.dma_start(out=xtc_sb[:, :256], in_=xtc[:, :256])
                        nc.sync.dma_start(out=wm_sbs[0][:, :256], in_=wm[0][:, :256])
                        nc.sync.dma_start(out=xtc_sb[:, 256:], in_=xtc[:, 256:])
                        nc.sync.dma_start(out=wm_sbs[0][:, 256:], in_=wm[0][:, 256:])
                        nc.sync.dma_start(out=wm_sbs[2][:, :], in_=wm[2])
                        inq2.dma_start(out=ytr_sb[:, :2 * 512], in_=ytr[:, :2 * 512])
                        inq2.dma_start(out=wm_sbs[1][:, :], in_=wm[1])
                        inq2.dma_start(out=ytr_sb[:, 2 * 512:], in_=ytr[:, 2 * 512:])
                        nc.sync.dma_start(out=xtr_sb[:, :], in_=xtr)
                        inq2.dma_start(out=wm_sbs[3][:, :], in_=wm[3])
                        nc.sync.dma_start(out=a_sb[:, :], in_=abig)
                        inq2.dma_start(out=tlb_all[:, :], in_=hh)

                hT_sb = cp.tile([P, 2 * 256], DT_MAIN, tag="hT", bufs=2)
                tT_sb = cp.tile([P, 2 * 512], DT_MAIN, tag="tT", bufs=2)
                headT_sb = cp.tile([P, 2 * 256], DT_MAIN, tag="headT", bufs=2)
                tailT_sb = cp.tile([P, 2 * 512], DT_MAIN, tag="tailT", bufs=2)

                # ---- stage A: the four MLPs, transposed layout ----
                # dc-major loop order (both jc accumulate per input chunk) so
                # each MLP consumes its rhs / weight chunks as they arrive.
                def mlp(wi, rhs_sb, nfree, dst_sb, func, alpha):
                    pt0 = psa.tile([P, 512], mybir.dt.float32, tag="tmp")
                    pt1 = psa.tile([P, 512], mybir.dt.float32, tag="tmp")
                    pts = [pt0, pt1]
                    if ka:
                        ka_mm(pt0)
                        ka_mm(pt1)
                    for dc in range(6):
                        for jc in range(2):
                            nc.tensor.matmul(
                                pts[jc][:, :nfree],
                                wm_sbs[wi][:, dc * 256 + jc * 128:dc * 256 + (jc + 1) * 128],
                                rhs_sb[:, dc * nfree:(dc + 1) * nfree],
                                start=(dc == 0), stop=(dc == 5))
                    for jc in range(2):
                        nc.scalar.activation(
                            dst_sb[:, jc * nfree:(jc + 1) * nfree], pts[jc][:, :nfree],
                            func, bias=bias_sb[:, wi * 2 + jc:wi * 2 + jc + 1], alpha=alpha)

                # g(kk) = A_k^T h, pipelined two classes ahead of the mains.
                g_sbs = {}

                def gstage(kk):
                    g_ps = psg.tile([P, 512], mybir.dt.float32, tag="g")
                    if ka:
                        ka_mm(g_ps)
                    for jh in range(2):
                        for ic in range(2):
                            nc.tensor.matmul(
                                g_ps[:, jh * 256:(jh + 1) * 256],
                                a_sb[:, ((kk * 2 + ic) * 2 + jh) * 128:((kk * 2 + ic) * 2 + jh + 1) * 128],
                                hT_sb[:, ic * 256:(ic + 1) * 256],
                                start=(ic == 0), stop=(ic == 1))
                    g_sb = wp.tile([P, 512], DT_MAIN, tag="gsb", bufs=4)
                    nc.scalar.copy(g_sb[:, :], g_ps[:, :])
                    g_sbs[kk] = g_sb

                mlp(0, xtc_sb, 256, hT_sb, GELU, 0.0)
                mlp(2, xtc_sb, 256, headT_sb, LRELU, 0.01)

                # ---- stage B (r first: packed rows must be ready before the
                # K=2 cv+r matmuls in stage C) ----
                # rT[kk, m] = h.u + head.Wh and cv[kk, n'] = t.v + tail.Wt,
                # each packed onto a single partition (rrow2[1] / cvo2[0])
                # whose partner partition holds host-loaded ones, so one K=2
                # matmul per (kk, mt) adds cv (ones x cv_row) AND
                # r (r_col x ones) into PSUM at the cost of the old K=1
                # broadcast alone.
                import dataclasses
                cvo2 = cp.tile([2, KH * 512], DT_MAIN, tag="cvo2", bufs=2)
                rrow2 = cp.tile([2, KH * 256], DT_MAIN, tag="rrow2", bufs=2)
                nc.scalar.dma_start(out=cvo2[1:2, :], in_=onesd)
                nc.scalar.dma_start(out=rrow2[0:1, :], in_=onesd[0:1, :KH * 256])

                r2T_ps = psa.tile([8, 256], mybir.dt.float32, tag="tmp")
                for mt in range(2):
                    for si, (ui, src) in enumerate([(0, hT_sb), (1, headT_sb)]):
                        for jc in range(2):
                            nc.tensor.matmul(
                                r2T_ps[:, mt * 128:(mt + 1) * 128],
                                uvw_sb[:, (ui * 2 + jc) * 8:(ui * 2 + jc) * 8 + 8],
                                src[:, jc * 256 + mt * 128:jc * 256 + (mt + 1) * 128],
                                start=(si == 0 and jc == 0), stop=(si == 1 and jc == 1))
                rr_sb = wp.tile([KH, 256], DT_MAIN, tag="rr", bufs=2)
                nc.scalar.copy(rr_sb[:, :], r2T_ps[0:KH, :])
                nc.gpsimd.dma_start(out=rrow2[1:2, :], in_=rr_sb[:, :])

                mlp(1, ytr_sb, 512, tT_sb, GELU, 0.0)
                mlp(3, xtr_sb, 512, tailT_sb, LRELU, 0.01)

                cv_ps = psa.tile([KH, 512], mybir.dt.float32, tag="tmp")
                for si, (ui, src, nf) in enumerate([(2, tT_sb, 512), (3, tailT_sb, 512)]):
                    for jc in range(2):
                        nc.tensor.matmul(
                            cv_ps[:, :],
                            uvw_sb[:, (ui * 2 + jc) * 8:(ui * 2 + jc) * 8 + KH],
                            src[:, jc * nf:(jc + 1) * nf],
                            start=(si == 0 and jc == 0), stop=(si == 1 and jc == 1))
                cv_sb = wp.tile([KH, 512], DT_MAIN, tag="cv", bufs=2)
                nc.vector.tensor_copy(cv_sb[:, :], cv_ps[:, :])
                nc.gpsimd.dma_start(out=cvo2[0:1, :], in_=cv_sb[:, :])

                gstage(0)
                gstage(1)

                # ---- stage C: per-class biaffine, g two classes ahead ----
                for kk in range(KH):
                    g_sb = g_sbs.pop(kk)
                    osb2 = wp.tile([P, 1024], DT_OUT, tag="osb", bufs=8)
                    if mtsplit:
                        # one 1-bank PSUM tile per m-half: the mt=0 half
                        # evacuates (and its bank frees) while the PE still
                        # streams the mt=1 matmuls.
                        m_pss = [psm.tile([P, 512], mybir.dt.float32, tag="m",
                                          name=f"m{kk}_{i}") for i in range(2)]
                    else:
                        m_ps = psm.tile([P, 1024], mybir.dt.float32, tag="m")
                        if ka:
                            ka_mm(m_ps)
                    for mt in range(2):
                        dst = (m_pss[mt][:, :] if mtsplit
                               else m_ps[:, mt * 512:(mt + 1) * 512])
                        for jh in range(2):
                            nc.tensor.matmul(
                                dst,
                                g_sb[:, jh * 256 + mt * 128:jh * 256 + (mt + 1) * 128],
                                tT_sb[:, jh * 512:(jh + 1) * 512],
                                start=(jh == 0), stop=False)
                        nc.tensor.matmul(
                            dst,
                            rrow2[0:2, kk * 256 + mt * 128:kk * 256 + (mt + 1) * 128],
                            cvo2[0:2, kk * 512:(kk + 1) * 512],
                            start=False, stop=True)
                        if mtsplit:
                            t512 = tlb_all[:, kk * 640 + mt * 128:
                                           kk * 640 + mt * 128 + 512]
                            nc.vector.tensor_tensor(
                                osb2[:, mt * 512:(mt + 1) * 512],
                                m_pss[mt][:, :], t512, ADD)
                    if kk + 2 < KH:
                        gstage(kk + 2)
                    if not mtsplit:
                        # fused evacuation: psum + toeplitz windows (both mt
                        # halves in one DVE op via an overlapping 3D AP)
                        t640 = tlb_all[:, kk * 640:(kk + 1) * 640]
                        tlb2 = dataclasses.replace(
                            t640, ap=[list(t640.ap[0]), [128, 2], [1, 512]])
                        nc.vector.tensor_tensor(
                            osb2[:, :].rearrange("p (mt n) -> p mt n", mt=2),
                            m_ps[:, :].rearrange("p (mt n) -> p mt n", mt=2),
                            tlb2, ADD)
                    # outputs never share a queue with the next body's
                    # big input loads (which ride sync and, for split_pool,
                    # also pool).
                    if qplan == "pool_out":
                        oq = nc.gpsimd
                    elif qplan == "alt_out":
                        oq = nc.gpsimd if kk % 2 == 0 else nc.scalar
                    else:  # act_out, split_pool
                        oq = nc.scalar
                    oq.dma_start(
                        out=out_d[kk].rearrange("mt p n -> p mt n"),
                        in_=osb2[:, :].rearrange("p (mt n) -> p mt n", mt=2))

            osb_const = None
            if mode in ("dma", "dmafew"):
                body = body_dma_only if mode == "dma" else body_dma_few
                osb_const = wp.tile([P, 1024], DT_OUT, tag="osb", bufs=1)
                nc.vector.memset(osb_const[:, :], 0.25)
            elif mode in ("compute", "both"):
                alloc_shared_inputs()
            warmup()
            if reps and reps > 0:
                # For_i has an all-engine barrier per iteration; unroll
                # UNROLL bodies per iteration (pools double-buffered) so
                # consecutive bodies software-pipeline across engines.
                assert reps % unroll == 0, f"reps must be divisible by {unroll}"
                with tc.For_i(0, reps // unroll, 1) as iv:
                    for _ in range(unroll):
                        body(iv)
            elif reps:
                for _ in range(-reps):   # unrolled, for steady-state sim
                    body()
            else:
                body()

    nc.compile()
    return nc


def _get_program(act_mode="hw", reps=0, mlp_dt=None, main_dt=None, mode="full",
                 unroll=None, split_in=False, qplan="pool_out", ka=False,
                 mtsplit=False, fewin=True):
    key = (act_mode, reps, mlp_dt or MLP_DT, main_dt or MAIN_DT, mode,
           unroll or UNROLL, split_in, qplan, ka, mtsplit, fewin)
    if key not in _nc:
        _nc[key] = _build_program(act_mode, reps, mlp_dt, main_dt, mode,
                                  unroll, split_in, qplan, ka, mtsplit, fewin)
    return _nc[key]


def _cast(a, dt_name):
    if dt_name == "f32r":
        return np.ascontiguousarray(a, dtype=np.float32)
    import ml_dtypes
    return np.ascontiguousarray(np.asarray(a, np.float32).astype(ml_dtypes.bfloat16))


def _pack_k(a):
    """(6*128, C) -> (128, 6*C): partition p gets contraction rows p, p+128, ..."""
    hid, c = a.shape
    dc = hid // P
    return np.ascontiguousarray(
        a.reshape(dc, P, c).transpose(1, 0, 2).reshape(P, dc * c))


def make_in_maps(x, y, mlp1_w, mlp1_b, mlp2_w, mlp2_b, head_w, head_b,
                 tail_w, tail_b, biaf_W, W, size_emb, mlp_dt=None, main_dt=None):
    import ml_dtypes
    mlp_dt = mlp_dt or MLP_DT
    main_dt = main_dt or MAIN_DT
    f = np.float32
    x = np.asarray(x, f)
    y = np.asarray(y, f)
    # (4, 128, 6*256): partition p holds HID rows {p, p+128, ...} of each W.T
    wm_all = _cast(np.stack([_pack_k(w.T.astype(f))
                             for w in (mlp1_w, mlp2_w, head_w, tail_w)]), mlp_dt)
    biasv4 = np.stack([mlp1_b, mlp2_b, head_b, tail_b]).astype(f)      # (4,256)
    # biasv[p, wi*2+jc] = b[wi][jc*128+p]
    biasv = np.ascontiguousarray(
        biasv4.reshape(4, 2, P).transpose(2, 0, 1).reshape(P, 8))

    Ws = W[:, 514:539]
    cval = W[:, 256] + W[:, 513] + biaf_W[:, 256, 256]                 # (14,)
    T0 = np.asarray((size_emb @ Ws.T).T + cval[:, None], f)            # (14,30)

    xT = {b: np.ascontiguousarray(x[b].T) for b in range(B)}           # (768,512)
    xTr = {b: x[b].T[:, ::-1] for b in range(B)}
    yTr = {b: y[b].T[:, ::-1] for b in range(B)}

    wprime = np.arange(767)
    ones_row = np.ones((1, KH * 512), np.float32)
    ones_row = _cast(ones_row, main_dt)
    in_maps = []
    for c in range(8):
        b, mh, khalf = c // 4, (c // 2) % 2, c % 2
        m0, k0 = mh * 256, khalf * KH
        ks = slice(k0, k0 + KH)
        uvw_m = np.zeros((4, 256, 8), f)
        uvw_m[0, :, :KH] = biaf_W[ks, :256, 256].T     # U    (256,7)
        uvw_m[1, :, :KH] = W[ks, :256].T               # WhT
        uvw_m[2, :, :KH] = biaf_W[ks, 256, :256].T     # V
        uvw_m[3, :, :KH] = W[ks, 257:513].T            # WtT
        # uvw[p, (wi*2+jc)*8+k] = uvw_m[wi, jc*128+p, k]
        uvw_p = _cast(uvw_m.reshape(4, 2, P, 8).transpose(2, 0, 1, 3).reshape(P, 64),
                      main_dt)
        # abig[p, ((k*2+ic)*2+jh)*128+c] = A[k, ic*128+p, jh*128+c]
        A = np.asarray(biaf_W[ks, :256, :256], f)
        a_p = _cast(A.reshape(KH, 2, P, 2, P).transpose(2, 0, 1, 3, 4)
                    .reshape(P, KH * 2 * 2 * P), main_dt)
        # hh[p, kk*640+w] = T0[k0+kk][clip(526 - (p + w + m0), 0, 29)]
        hh_rows = np.stack(
            [T0[k0 + kk][np.clip(526 - (wprime + m0), 0, 29)] for kk in range(KH)]
        )                                                               # (7,767)
        idx = np.arange(P)[:, None] + np.arange(640)[None, :]           # (128,640)
        hh_m = np.ascontiguousarray(
            hh_rows[:, idx].transpose(1, 0, 2).reshape(P, KH * 640)
            .astype(ml_dtypes.float8_e4m3))
        in_maps.append({
            "wm": wm_all,
            "onesd": ones_row,
            "xtc": _cast(_pack_k(xT[b][:, m0:m0 + 256]), mlp_dt),
            "ytr": _cast(_pack_k(yTr[b]), mlp_dt),
            "xtr": _cast(_pack_k(xTr[b]), mlp_dt),
            "abig": a_p,
            "uvw": uvw_p,
            "biasv": biasv,
            "hh": hh_m,
        })
    return in_maps


def assemble(results):
    out = np.empty((B, CLS, N, N), np.float32)
    for c, r in enumerate(results):
        b, mh, khalf = c // 4, (c // 2) % 2, c % 2
        blk = np.asarray(r["out"], np.float32).reshape(KH, 256, 512)[:, :, ::-1]
        out[b, khalf * KH:(khalf + 1) * KH, mh * 256:(mh + 1) * 256, :] = blk
    return out


def kernel(**inputs):
    from concourse import bass_utils
    inputs = {k: np.asarray(v) for k, v in inputs.items()}
    nc = _get_program()
    in_maps = make_in_maps(
        inputs["x"], inputs["y"],
        inputs["mlp1_w"], inputs["mlp1_b"], inputs["mlp2_w"], inputs["mlp2_b"],
        inputs["head_w"], inputs["head_b"], inputs["tail_w"], inputs["tail_b"],
        inputs["biaf_W"], inputs["W"], inputs["size_emb"])
    res = bass_utils.run_bass_kernel_spmd(nc, in_maps, core_ids=list(range(8)),
                                          trace=False)
    return assemble(res.results)

